# revision 9
# baseline (speedup 1.0000x reference)
"""Trainium2 Bass kernel for nn_BasicNCAModel (neural cellular automaton).

Data parallel: batch 16 -> 2 images per core x 8 cores.  ~0.95 ms traced
(baseline 1.65 ms), rel err 1.6e-2 (gate 2e-2; inputs are deterministic
and hardware matches the CPU quantization emulator to 5 digits).

Design:
* The two depthwise 3x3 convs never materialize: they are folded into
  the hidden-layer matmul, h = relu(sum_tap E_tap @ x_shift(tap) + b)
  with host-precomputed E_tap[256, 64]; taps accumulate in PSUM, the
  two images running as concurrent PE row-tiles 0-63 / 64-127.
* Mixed precision mm1: the 5 cross/center taps run fp16 (fast weight
  load, 10 mantissa bits); the 4 corner taps run fp8e4m3 packed as TWO
  DoubleRow matmuls -- pairs (a,0)+(a,2) share one instruction via an
  overlapping-window rhs AP ([K, 2, RPG, H], dim1 element-step 2) and
  2-wide interleaved weights, halving their PE cost.  Corner taps carry
  only ~27% of the mm1 variance, so the fp8 error stays in budget
  (CPU-emulated rel 0.016 vs 0.026 for all-fp8, which fails).
* An fp8 shadow of the state (ping-pong + its own reflect halos) is
  maintained by GpSimd casts in the tail; mm1 at t=0 reads separate
  raw-x0 tiles (fp16 + fp8).
* Stochastic fire gate x static life mask pre-broadcast on the host to
  [128, NPIX] fp8e4 and DMA'd per group; life-mask folded into the
  uploaded initial state (x0*life) so t=0 needs no special multiplies.
* Reflect-pad halos maintained per group (GpSimd); the state-update
  tail lags its group by up to 4 groups and runs across step
  boundaries -- no per-step barriers.  Tails are batched 4 groups at a
  time so the PE row-tile -> col-tile weight turnaround (mm2 is
  col-tiled M=64+64) is paid once per batch.
* Startup: input DMAs split across the sync/scalar HWDGE rings with the
  group-0 critical path (first state rows fp16+fp8, m0 weights, DR
  weights) ahead of the bulk.
"""
import sys
sys.path.insert(0, '/opt/trn_rl_repo')

import numpy as np

B, H, W, C = 16, 128, 128, 64
HID = 256
STEPS = 8
NCORES = 8
BPC = B // NCORES            # images per core = 2
WP, HP = W + 2, H + 2        # padded grid 130 x 130
RPG = 4                      # W-rows per group
NPIX = RPG * H               # 512 pixels per matmul tile
NG = W // RPG                # 32 groups per step

_nc_cache = {}


def _build():
    from concourse import bass as bassmod
    import concourse.bacc as bacc
    import concourse.mybir as mybir
    import concourse.tile as tile

    F32 = mybir.dt.float32
    F16 = mybir.dt.float16
    F8 = mybir.dt.float8e4
    AF = mybir.ActivationFunctionType
    ALU = mybir.AluOpType

    nc = bacc.Bacc("TRN2", target_bir_lowering=False, debug=False,
                   enable_asserts=False, num_devices=NCORES)

    XM = nc.dram_tensor("xm", [128, WP, HP], F16, kind="ExternalInput")
    XR = nc.dram_tensor("xr", [128, WP, HP], F16, kind="ExternalInput")
    XR8 = nc.dram_tensor("xr8", [128, WP, HP], F8, kind="ExternalInput")
    WDR = nc.dram_tensor("wdr", [128, 2, 3, 2, 128], F8, kind="ExternalInput")
    WT = nc.dram_tensor("wt", [128, 2, 9, 128], F16, kind="ExternalInput")
    WF = nc.dram_tensor("wf", [128, 2, 64], F16, kind="ExternalInput")
    BI = nc.dram_tensor("bi", [128, 2], F32, kind="ExternalInput")
    GL = nc.dram_tensor("gl", [STEPS, NG, 128, NPIX], F8, kind="ExternalInput")
    OUT = nc.dram_tensor("out", [128, W, H], F16, kind="ExternalOutput")

    with tile.TileContext(nc) as tc:
        with tc.tile_pool(name="const", bufs=1) as cp, \
             tc.tile_pool(name="hbuf", bufs=5, space="SBUF") as hp, \
             tc.tile_pool(name="gbuf", bufs=6) as gp, \
             tc.tile_pool(name="ph", bufs=1, space="PSUM") as php, \
             tc.tile_pool(name="pdx", bufs=1, space="PSUM") as pdxp:

            xA = cp.tile([128, WP, HP], F16, tag="xA")
            xB = cp.tile([128, WP, HP], F16, tag="xB")
            xr = cp.tile([128, WP, HP], F16, tag="xr")
            y8A = cp.tile([128, WP, HP], F8, tag="y8A")
            y8B = cp.tile([128, WP, HP], F8, tag="y8B")
            y8r = cp.tile([128, WP, HP], F8, tag="y8r")
            wdr = cp.tile([128, 2, 3, 2, 128], F8, tag="wdr")
            wt = cp.tile([128, 2, 9, 128], F16, tag="wt")
            wf = cp.tile([128, 2, 64], F16, tag="wf")
            bi = cp.tile([128, 2], F32, tag="bi")

            # critical path: group-0 mm1 needs xr rows 0-7 and wt only; put
            # those first on the sync ring, everything else on the scalar
            # HWDGE ring (parallel), bulk xr behind on sync.
            nc.sync.dma_start(xr[:, 0:8, :], XR[:, 0:8, :])
            nc.sync.dma_start(wt[:, 0], WT[:, 0])
            # one transfer: the DoubleRow rhs APs are hand-built, which gets
            # a conservative whole-tensor dependency -- so y8r must land as a
            # single early DMA or the in-order PE FIFO stalls on the first DR
            nc.sync.dma_start(y8r[:], XR8[:])
            for r0, r1 in [(8, 50), (50, 90), (90, 130)]:
                nc.sync.dma_start(xr[:, r0:r1, :], XR[:, r0:r1, :])
            nc.scalar.dma_start(bi[:], BI[:])
            nc.scalar.dma_start(wdr[:], WDR[:])
            nc.scalar.dma_start(wt[:, 1], WT[:, 1])
            nc.scalar.dma_start(wf[:], WF[:])
            for c in range(4):
                r0, r1 = (WP * c) // 4, (WP * (c + 1)) // 4
                nc.scalar.dma_start(xA[:, r0:r1, :], XM[:, r0:r1, :])

            SINGLE = [(1, 0, 1), (4, 1, 1), (7, 2, 1)]
            DRPAIRS = [(0, 0), (1, 1), (2, 2)]  # (pair idx, row a): (a,0)+(a,2)
            DRMODE = mybir.MatmulPerfMode.DoubleRow

            def emit_tails(ps):
                """mm2 + gate-multiply + state update for one or two completed
                groups, batched so the row->col->row PE weight-tile turnaround
                is paid once per pair instead of once per group."""
                dxs = []
                for i, p in enumerate(ps):
                    hA, hB, gt, xadd, xd, y8d, w0, last = p
                    dx = pdxp.tile([128, NPIX], F32, tag=f"dx{i}",
                                   name=f"dx{i}")
                    dxs.append(dx)
                    for k in range(2):
                        nc.tensor.matmul(dx[0:64, :], wf[:, k, :], hA[:, k, :],
                                         start=k == 0, stop=k == 1,
                                         skip_group_check=True)
                        nc.tensor.matmul(dx[64:128, :], wf[:, k, :], hB[:, k, :],
                                         start=k == 0, stop=k == 1,
                                         skip_group_check=True,
                                         tile_position=(0, 64))
                for p, dx in zip(ps, dxs):
                    hA, hB, gt, xadd, xd, y8d, w0, last = p
                    tg = hp.tile([128, NPIX], F16, tag="tg")
                    nc.vector.tensor_tensor(tg[:], dx[:], gt[:], ALU.mult)
                    tg3 = tg[:].rearrange("p (a b) -> p a b", a=RPG)
                    src_i = xadd[:, w0 + 1:w0 + 1 + RPG, 1:1 + H]
                    dst_i = xd[:, w0 + 1:w0 + 1 + RPG, 1:1 + H]
                    nc.vector.tensor_tensor(dst_i, tg3, src_i, ALU.add)
                    if last:
                        nc.sync.dma_start(OUT[:, w0:w0 + RPG, :], dst_i)
                        continue
                    # fp8 shadow copy of the updated rows for the DR taps
                    nc.gpsimd.tensor_copy(y8d[:, w0 + 1:w0 + 1 + RPG, 1:1 + H],
                                          dst_i)
                    # reflect halo for the rows just written (cols first, then
                    # the top/bottom halo row once its source row is complete)
                    nc.gpsimd.tensor_copy(xd[:, w0 + 1:w0 + 1 + RPG, 0],
                                          xd[:, w0 + 1:w0 + 1 + RPG, 2])
                    nc.gpsimd.tensor_copy(xd[:, w0 + 1:w0 + 1 + RPG, HP - 1],
                                          xd[:, w0 + 1:w0 + 1 + RPG, HP - 3])
                    nc.gpsimd.tensor_copy(y8d[:, w0 + 1:w0 + 1 + RPG, 0],
                                          y8d[:, w0 + 1:w0 + 1 + RPG, 2])
                    nc.gpsimd.tensor_copy(y8d[:, w0 + 1:w0 + 1 + RPG, HP - 1],
                                          y8d[:, w0 + 1:w0 + 1 + RPG, HP - 3])
                    if w0 == 0:
                        nc.gpsimd.tensor_copy(xd[:, 0, :], xd[:, 2, :])
                        nc.gpsimd.tensor_copy(y8d[:, 0, :], y8d[:, 2, :])
                    if w0 == W - RPG:
                        nc.gpsimd.tensor_copy(xd[:, WP - 1, :], xd[:, WP - 3, :])
                        nc.gpsimd.tensor_copy(y8d[:, WP - 1, :], y8d[:, WP - 3, :])

            pends = []
            for t in range(STEPS):
                xs, xd = (xA, xB) if t % 2 == 0 else (xB, xA)
                y8s, y8d = (y8A, y8B) if t % 2 == 0 else (y8B, y8A)
                xmm = xr if t == 0 else xs
                y8mm = y8r if t == 0 else y8s
                for g in range(NG):
                    w0 = RPG * g

                    gt = gp.tile([128, NPIX], F8, tag="gt")
                    nc.sync.dma_start(gt[:], GL[t, g])

                    # mm1: folded conv + hidden layer, 9 taps x 2 M-chunks,
                    # images A/B as concurrent PE row-tiles; tails for the
                    # previous TWO groups slot in between the two m-chunks.
                    phs = [[php.tile([128, NPIX], F32, tag=f"ph{im}{m}",
                                     name=f"ph{im}{m}")
                            for m in range(2)] for im in range(2)]
                    hA = hp.tile([128, 2, NPIX], F16, tag="hA")
                    hB = hp.tile([128, 2, NPIX], F16, tag="hB")
                    for m in range(2):
                        for si, (ti, a, b) in enumerate(SINGLE):
                            rhsA = xmm[0:64, w0 + a:w0 + a + RPG, b:b + H]
                            rhsB = xmm[64:128, w0 + a:w0 + a + RPG, b:b + H]
                            st = si == 0
                            nc.tensor.matmul(phs[0][m][:], wt[0:64, m, ti, :], rhsA,
                                             start=st, stop=False, skip_group_check=True)
                            nc.tensor.matmul(phs[1][m][:], wt[64:128, m, ti, :], rhsB,
                                             start=st, stop=False, skip_group_check=True)
                        # corner taps (a,0)+(a,2) as fp8 DoubleRow pairs: the
                        # two b-shifts pack as rhs dim1 with element step 2
                        for pi, a in DRPAIRS:
                            sp = pi == 2
                            for im, (p0, p1) in enumerate(((0, 64), (64, 128))):
                                base = y8mm[p0:p1, w0 + a:w0 + a + RPG, 0:H]
                                rhs8 = bassmod.AP(
                                    base.tensor, offset=base.offset,
                                    ap=[list(base.ap[0]), [2, 2],
                                        list(base.ap[1]), list(base.ap[2])])
                                nc.tensor.matmul(phs[im][m][:],
                                                 wdr[p0:p1, m, pi, :, :], rhs8,
                                                 start=False, stop=sp,
                                                 skip_group_check=True,
                                                 perf_mode=DRMODE)
                        # relu + bias for this chunk, PSUM -> SBUF (f16 for mm2)
                        nc.scalar.activation(hA[:, m, :], phs[0][m][:], AF.Relu,
                                             bias=bi[:, m:m + 1])
                        nc.scalar.activation(hB[:, m, :], phs[1][m][:], AF.Relu,
                                             bias=bi[:, m:m + 1])
                        if m == 0 and len(pends) == 4:
                            emit_tails(pends)
                            pends = []

                    pends.append((hA, hB, gt, xs, xd, y8d, w0, t == STEPS - 1))

            # flush the remaining tails (the tail-lag pipeline runs across
            # step boundaries: tail(g31, t) only writes rows 125-128, which
            # step t+1 reads last, so the lag is safe).
            emit_tails(pends)
            pends = []

    nc.compile()
    return nc


def _host_pack(x, w_conv1, w_conv2, w_hidden, b_hidden, w_final, rand_vals):
    import ml_dtypes
    f8 = ml_dtypes.float8_e4m3

    Wh = np.asarray(w_hidden, np.float64)            # [256, 192]
    w1 = np.asarray(w_conv1, np.float64)[:, 0]       # [64, 3, 3]
    w2 = np.asarray(w_conv2, np.float64)[:, 0]

    wtaps = np.zeros((128, 2, 9, 128), np.float32)
    for ti, (a, b) in enumerate([(a, b) for a in range(3) for b in range(3)]):
        E = Wh[:, 64:128] * w1[None, :, a, b] + Wh[:, 128:192] * w2[None, :, a, b]
        if (a, b) == (1, 1):
            E = E + Wh[:, 0:64]
        for m in range(2):
            lhsT = E[128 * m:128 * (m + 1), :].T.astype(np.float32)   # [64, 128]
            wtaps[0:64, m, ti, :] = lhsT
            wtaps[64:128, m, ti, :] = lhsT
    wtaps = wtaps.astype(np.float16)

    wfz = np.asarray(w_final, np.float32).copy()     # [64, 256]
    wfz[0:4, :] = 0.0                                # immutable image channels
    wfT = wfz.T                                      # [256, 64]
    wf = np.stack([wfT[0:128], wfT[128:256]], axis=1)          # [128, 2, 64]
    wf = np.ascontiguousarray(wf).astype(np.float16)

    bi = np.stack([b_hidden[0:128], b_hidden[128:256]], axis=1).astype(np.float32)

    # life mask is static: channel-0 updates are masked out, so
    # life(t) == (x0_init > 0) for every step
    Lhw = np.asarray(x)[..., 0] > 0                  # [B, H, W]
    Lwh = np.ascontiguousarray(Lhw.transpose(0, 2, 1))   # [B, W, H]
    G = np.asarray(rand_vals)[..., 0] > 0.5          # [S, B, H, W]
    GLw = (G.transpose(0, 1, 3, 2) & Lwh[None]).astype(np.float32)  # [S,B,W,H]

    x_chw = np.asarray(x, np.float32).transpose(0, 3, 2, 1)      # [B, C, W, H]
    xp_raw = np.pad(x_chw, ((0, 0), (0, 0), (1, 1), (1, 1)), mode='reflect')
    x_msk = x_chw * Lwh[:, None, :, :]               # x0 * life
    xp_msk = np.pad(x_msk, ((0, 0), (0, 0), (1, 1), (1, 1)), mode='reflect')

    wdr = np.zeros((128, 2, 3, 2, 128), np.float32)
    for pi, a in enumerate((0, 1, 2)):
        for ii, b in enumerate((0, 2)):
            E = Wh[:, 64:128] * w1[None, :, a, b] + Wh[:, 128:192] * w2[None, :, a, b]
            for m in range(2):
                lhsT = E[128 * m:128 * (m + 1), :].T.astype(np.float32)
                wdr[0:64, m, pi, ii, :] = lhsT
                wdr[64:128, m, pi, ii, :] = lhsT
    wdr = wdr.astype(f8)

    in_maps = []
    for i in range(NCORES):
        sl = slice(BPC * i, BPC * (i + 1))
        xrc = np.ascontiguousarray(
            xp_raw[sl].reshape(BPC * C, WP, HP)).astype(np.float16)
        xr8c = np.ascontiguousarray(
            xp_raw[sl].reshape(BPC * C, WP, HP)).astype(f8)
        xmc = np.ascontiguousarray(
            xp_msk[sl].reshape(BPC * C, WP, HP)).astype(np.float16)
        # gates: [S, NG, 128, NPIX], partitions = img*64 + channel
        glc = GLw[:, sl].reshape(STEPS, BPC, NG, NPIX).transpose(0, 2, 1, 3)
        glc = np.ascontiguousarray(
            np.repeat(glc, C, axis=2)).astype(f8)     # [S, NG, 128, NPIX]
        in_maps.append({
            "xm": xmc, "xr": xrc, "xr8": xr8c, "wt": wtaps, "wf": wf,
            "bi": bi, "gl": glc, "wdr": wdr,
        })
    return in_maps


def _run(inputs, trace=False, trace_kwargs=None):
    from concourse.bass_utils import run_bass_kernel_spmd
    if "nc" not in _nc_cache:
        _nc_cache["nc"] = _build()
    nc = _nc_cache["nc"]
    in_maps = _host_pack(
        inputs["x"], inputs["w_conv1"], inputs["w_conv2"], inputs["w_hidden"],
        inputs["b_hidden"], inputs["w_final"], inputs["rand_vals"])
    kwargs = {}
    if trace:
        kwargs["trace"] = True
        if trace_kwargs:
            kwargs.update(trace_kwargs)
    res = run_bass_kernel_spmd(nc, in_maps, core_ids=list(range(NCORES)), **kwargs)
    outs = []
    for i in range(NCORES):
        o = res.results[i]["out"].astype(np.float32).reshape(BPC, C, W, H)
        outs.append(o.transpose(0, 3, 2, 1))         # -> [b, H, W, C]
    full = np.concatenate(outs, axis=0).astype(np.float32)
    return full, res


def kernel(**inputs) -> np.ndarray:
    steps = int(np.asarray(inputs.get("steps", STEPS)))
    assert steps == STEPS, f"kernel compiled for {STEPS} steps, got {steps}"
    out, _ = _run(inputs)
    return out
